# revision 10
# baseline (speedup 1.0000x reference)
"""Blended-expert MoE MLP (moe_routing) Trainium2 Bass kernel.

Math per layer l:  t[b,o] = sum_e wb[b,e] * (W_l[e] @ x[b] + B_l[e])
                   x_next = elu(t)   (layers 0,1; layer 2 linear)

Reformulated as one GEMM per layer with contraction k = (i_tile, e, p):
    t[o, b] = sum_k Wp[k, o] * xp[k, b]
where xp[(i_tile,e,p), b] = xT[i_tile*128+p, b] * wbT[e, b]  (built on-chip
by DVE) and the bias enters as an extra K=8 matmul with rhs = wbT directly.

v2: skewed per-output-pair contraction. Weights are laid out host-side as
[o_pair j][k-chunk c][g][p][256 o] so each 512KB chunk is contiguous in DRAM
and streams in exactly the order the PE consumes it. Each o-pair's PSUM
accumulation closes after its KT k-tiles, so evict+ELU+next-layer xp build
overlap the next pair's matmuls, and layer l+1's matmuls overlap layer l's
tail pairs (cross-layer software pipelining via Tile dependency scheduling).
PSUM tiles are padded to a full 2KB bank each to avoid bank-sharing hazards.

Everything on-device is feature-major ([feature, batch]) so each layer's
PSUM output [o, b] is directly the next layer's input layout.

Sharding: data-parallel over batch: 2048 -> 8 cores x 256. Weights are
replicated (streamed from HBM each layer, ~33MB/core in f16).
"""

import os
import sys

import numpy as np

if not any("trn_rl_repo" in p for p in sys.path):
    sys.path.append("/opt/trn_rl_repo")

from concourse import bacc, mybir  # noqa: E402
import concourse.bass as bass  # noqa: E402
import concourse.tile as tile  # noqa: E402

F32 = mybir.dt.float32
F16 = mybir.dt.float16


def _mm_mode():
    return os.environ.get("MOE_MM_DTYPE", "f16")


E = 8
DIMS = [512, 1024, 1024, 512]
BATCH = 2048
NCORES = 8
B = BATCH // NCORES  # 256 per-core batch
P = 128
OW = 512  # o-columns per group (4 PSUM tiles; 1KB DMA runs)
TPG = OW // P  # tiles per group

NI = [DIMS[0] // P, DIMS[1] // P, DIMS[2] // P]  # [4, 8, 8] input tiles / layer
NO = [DIMS[1] // P, DIMS[2] // P, DIMS[3] // P]  # [8, 8, 4] output tiles / layer
NJ = [DIMS[1] // OW, DIMS[2] // OW, DIMS[3] // OW]  # [2, 2, 1] o-groups / layer
KT = [NI[l] * E for l in range(3)]  # [32, 64, 64] contraction tiles / layer
G = 4  # k-tiles per streamed weight chunk

_CACHE = {}


def _build_program(mm_mode: str = "f16", reps: int = 1, hw_loop: int = 0):
    """Build (and cache) the Bass program. Same program runs SPMD on all cores.
    reps>1 unrolls the whole computation in-program; hw_loop>0 wraps it in a
    hardware For_i loop (for timing-slope measurements that cancel out
    per-dispatch overhead)."""
    key = ("prog", mm_mode, reps, hw_loop)
    if key in _CACHE:
        return _CACHE[key]
    assert mm_mode == "f16", "v2 kernel supports f16 only"

    nc = bacc.Bacc("TRN2", target_bir_lowering=False, debug=False, num_devices=NCORES)

    xT_d = nc.dram_tensor("xT", [DIMS[0], B], F32, kind="ExternalInput")
    wbT16_d = nc.dram_tensor("wbT16", [E, B], F16, kind="ExternalInput")
    wp_d = [
        nc.dram_tensor(f"Wp{l}", [NJ[l] * KT[l] * P, OW], F16, kind="ExternalInput")
        for l in range(3)
    ]
    wb_d = [
        nc.dram_tensor(f"Wb{l}", [E, DIMS[l + 1]], F16, kind="ExternalInput")
        for l in range(3)
    ]
    yT_d = nc.dram_tensor("yT", [DIMS[3], B], F32, kind="ExternalOutput")

    stag = os.environ.get("MOE_STAGGER", "0") == "1"

    with tile.TileContext(nc) as tc:
        with (
            tc.tile_pool(name="const", bufs=1) as const_pool,
            tc.tile_pool(name="xpool", bufs=2) as x_pool,
            tc.tile_pool(name="xppool", bufs=2) as xp_pool,
            tc.tile_pool(name="wstream", bufs=6) as w_pool,
            tc.tile_pool(name="tmp", bufs=8) as tmp_pool,
            tc.tile_pool(name="psum", bufs=8, space="PSUM") as psum_pool,
        ):
            # ---- loop-invariant constants: blend rows, blend broadcast, biases ----
            wb_sb = const_pool.tile([E, B], F16)
            nc.sync.dma_start(wb_sb[:], wbT16_d[:])

            # wb broadcast to all 128 partitions: [128, E, B] (f16, 512KB)
            wb_bc = const_pool.tile([P, E, B], F16)
            nc.sync.dma_start(
                wb_bc[:],
                wbT16_d.rearrange("e b -> (e b)")
                .unsqueeze(0)
                .partition_broadcast(P)
                .squeeze(1)
                .rearrange("p (e b) -> p e b", e=E),
            )

            wbias = []
            for l in range(3):
                wbl = const_pool.tile([E, DIMS[l + 1]], F16, name=f"wbias_{l}")
                nc.sync.dma_start(wbl[:], wb_d[l][:])
                wbias.append(wbl)

            # x0 / xp0 are software-pipelined one iteration ahead: loaded and
            # built here for the first iteration, then re-loaded/rebuilt at
            # each iteration's tail (when DMA + DVE are otherwise idle) so the
            # PE can start layer 0 immediately after the loop barrier.
            x0_sb = const_pool.tile([P, NI[0], B], F32, name="x0")
            xp0_sb = const_pool.tile([P, KT[0], B], F16, name="xp0")

            def load_x0_build_xp0():
                h = NI[0] // 2
                nc.sync.dma_start(
                    x0_sb[:, :h, :],
                    xT_d[: h * P, :].rearrange("(t p) b -> p t b", p=P),
                )
                nc.sync.dma_start(
                    x0_sb[:, h:, :],
                    xT_d[h * P :, :].rearrange("(t p) b -> p t b", p=P),
                )
                for it in range(NI[0]):
                    nc.vector.tensor_tensor(
                        out=xp0_sb[:, it * E : (it + 1) * E, :],
                        in0=x0_sb[:, it : it + 1, :].broadcast_to([P, E, B]),
                        in1=wb_bc[:],
                        op=mybir.AluOpType.mult,
                    )

            load_x0_build_xp0()

            import contextlib

            loop_cm = (
                tc.For_i(0, hw_loop, 1, staggered_reset=stag)
                if hw_loop > 0
                else contextlib.nullcontext()
            )
            with loop_cm:
              for rep in range(reps):
                xp = xp0_sb
                wdma_n = 0
                for l in range(3):
                    nI, nJ, O = NI[l], NJ[l], DIMS[l + 1]
                    ktl = KT[l]
                    nchunk = ktl // G

                    if l < 2:
                        x_next = x_pool.tile(
                            [P, NO[l], B], F32, tag="x", name=f"x{l + 1}_{rep}"
                        )
                        xp_next = xp_pool.tile(
                            [P, KT[2], B], F16, tag="xp", name=f"xp{l + 1}_{rep}"
                        )
                    else:
                        x_next = x_pool.tile(
                            [P, NO[l], B], F32, tag="x", name=f"y_{rep}"
                        )
                        xp_next = None

                    for j in range(nJ):
                        # PSUM accumulators, one full bank per o-tile
                        po = []
                        for t in range(TPG):
                            po_t = psum_pool.tile(
                                [P, B], F32, tag="po",
                                name=f"po_{l}_{j}_{t}_{rep}",
                                padded_shape=[P, 512],
                            )
                            po.append(po_t)
                            # bias matmul opens the accumulation group
                            nc.tensor.matmul(
                                po_t[:],
                                wbias[l][:, j * OW + t * P : j * OW + (t + 1) * P],
                                wb_sb[:],
                                start=True,
                                stop=False,
                            )

                        # stream this group's weights in G-k-tile chunks,
                        # alternating the issuing HWDGE engine (SP / Activation)
                        for c in range(nchunk):
                            w_sb = w_pool.tile(
                                [P, G, OW], F16, tag="w", name=f"w_{l}_{j}_{c}_{rep}"
                            )
                            base = (j * ktl + c * G) * P
                            eng = nc.sync if (wdma_n % 2 == 0) else nc.scalar
                            wdma_n += 1
                            eng.dma_start(
                                w_sb[:],
                                wp_d[l][base : base + G * P, :].rearrange(
                                    "(g p) o -> p g o", p=P
                                ),
                            )
                            for g in range(G):
                                kt = c * G + g
                                last = kt == ktl - 1
                                for t in range(TPG):
                                    nc.tensor.matmul(
                                        po[t][:],
                                        w_sb[:, g, t * P : (t + 1) * P],
                                        xp[:, kt, :],
                                        start=False,
                                        stop=last,
                                    )

                        # ---- evict + ELU, then build next layer's xp slices ----
                        for t in range(TPG):
                            ot = TPG * j + t
                            if l < 2:
                                # elu(t) = (min(exp(t),1) - 1) + max(t, 0)
                                ex = tmp_pool.tile(
                                    [P, B], F32, tag="ex", name=f"ex_{l}_{ot}_{rep}"
                                )
                                nc.scalar.activation(
                                    ex[:], po[t][:], mybir.ActivationFunctionType.Exp
                                )
                                em1 = tmp_pool.tile(
                                    [P, B], F32, tag="em1", name=f"em1_{l}_{ot}_{rep}"
                                )
                                nc.vector.tensor_scalar(
                                    em1[:],
                                    ex[:],
                                    1.0,
                                    -1.0,
                                    op0=mybir.AluOpType.min,
                                    op1=mybir.AluOpType.add,
                                )
                                nc.vector.scalar_tensor_tensor(
                                    x_next[:, ot, :],
                                    po[t][:],
                                    0.0,
                                    em1[:],
                                    op0=mybir.AluOpType.max,
                                    op1=mybir.AluOpType.add,
                                )
                                # next layer xp slice (needs only this x tile)
                                nc.vector.tensor_tensor(
                                    out=xp_next[:, ot * E : (ot + 1) * E, :],
                                    in0=x_next[:, ot : ot + 1, :].broadcast_to(
                                        [P, E, B]
                                    ),
                                    in1=wb_bc[:],
                                    op=mybir.AluOpType.mult,
                                )
                            else:
                                # final layer: copy (no ELU), alternate engines,
                                # store each tile as soon as it's evicted
                                if t % 2 == 0:
                                    nc.vector.tensor_copy(x_next[:, ot, :], po[t][:])
                                else:
                                    nc.scalar.activation(
                                        x_next[:, ot, :],
                                        po[t][:],
                                        mybir.ActivationFunctionType.Copy,
                                    )
                                nc.sync.dma_start(
                                    yT_d[ot * P : (ot + 1) * P, :],
                                    x_next[:, ot, :],
                                )

                    if l < 2:
                        x_sb = x_next
                        xp = xp_next

                # prefetch next iteration's x0 / xp0 while DMA+DVE are idle
                load_x0_build_xp0()

    nc.compile()
    _CACHE[key] = nc
    return nc


def _prep_weights(W, l):
    """Rearrange (E, O, I) weights into the streamed layout:
    [j, kt, p, o256] with kt = i_tile*E + e, flattened to
    [nJ*KT*128, 256]; element (j, it*E+e, p, o) = W[e, j*256+o, it*128+p]."""
    O, I = DIMS[l + 1], DIMS[l]
    nI, nJ = I // P, O // OW
    out = (
        W.reshape(E, nJ, OW, nI, P)
        .transpose(1, 3, 0, 4, 2)
        .reshape(nJ * nI * E * P, OW)
    )
    return np.ascontiguousarray(out, dtype=np.float16)


def _prep_in_maps(weight_blend, x, W0, B0, W1, B1, W2, B2):
    weight_blend = np.asarray(weight_blend, dtype=np.float32)
    x = np.asarray(x, dtype=np.float32)
    Ws = [np.asarray(w, dtype=np.float32) for w in (W0, W1, W2)]
    Bs = [np.asarray(b, dtype=np.float32) for b in (B0, B1, B2)]
    wp = [_prep_weights(Ws[l], l) for l in range(3)]
    wbias = [np.ascontiguousarray(Bs[l][:, :, 0], dtype=np.float16) for l in range(3)]
    in_maps = []
    for c in range(NCORES):
        sl = slice(c * B, (c + 1) * B)
        in_maps.append(
            {
                "xT": np.ascontiguousarray(x[sl].T),
                "wbT16": np.ascontiguousarray(weight_blend[sl].T, dtype=np.float16),
                "Wp0": wp[0],
                "Wp1": wp[1],
                "Wp2": wp[2],
                "Wb0": wbias[0],
                "Wb1": wbias[1],
                "Wb2": wbias[2],
            }
        )
    return in_maps


def kernel(weight_blend, x, W0, B0, W1, B1, W2, B2):
    from concourse.bass_utils import run_bass_kernel_spmd

    in_maps = _prep_in_maps(weight_blend, x, W0, B0, W1, B1, W2, B2)
    nc = _build_program(mm_mode=_mm_mode())
    res = run_bass_kernel_spmd(nc, in_maps, list(range(NCORES)))
    out = np.concatenate([res.results[c]["yT"] for c in range(NCORES)], axis=1)
    return np.ascontiguousarray(out.T, dtype=np.float32)


def _make_sharded_fn(nc):
    """Build the shard_map'd jitted executable, mirroring
    bass2jax.run_bass_via_pjrt's multi-core path but without output donation
    so it can be re-invoked for timing."""
    import jax
    from jax.experimental.shard_map import shard_map
    from jax.sharding import Mesh, PartitionSpec
    from concourse import bass2jax, mybir as _mybir

    bass2jax.install_neuronx_cc_hook()

    partition_name = nc.partition_id_tensor.name if nc.partition_id_tensor else None
    in_names, out_names, out_avals, zero_outs = [], [], [], []
    for alloc in nc.m.functions[0].allocations:
        if not isinstance(alloc, _mybir.MemoryLocationSet):
            continue
        name = alloc.memorylocations[0].name
        if alloc.kind == "ExternalInput":
            if name != partition_name:
                in_names.append(name)
        elif alloc.kind == "ExternalOutput":
            out_names.append(name)
            shape = tuple(alloc.tensor_shape)
            dtype = _mybir.dt.np(alloc.dtype)
            out_avals.append(jax.core.ShapedArray(shape, dtype))
            zero_outs.append(np.zeros(shape, dtype))
    n_params = len(in_names)
    all_names = in_names + out_names
    if partition_name is not None:
        all_names = all_names + [partition_name]

    def _body(*args):
        operands = list(args)
        if partition_name is not None:
            operands.append(bass2jax.partition_id_tensor())
        outs = bass2jax._bass_exec_p.bind(
            *operands,
            out_avals=tuple(out_avals),
            in_names=tuple(all_names),
            out_names=tuple(out_names),
            lowering_input_output_aliases=(),
            sim_require_finite=True,
            sim_require_nnan=True,
            nc=nc,
        )
        return tuple(outs)

    devices = jax.devices()[:NCORES]
    mesh = Mesh(np.asarray(devices), ("core",))
    n_all = n_params + len(out_names)
    sharded = jax.jit(
        shard_map(
            _body,
            mesh=mesh,
            in_specs=(PartitionSpec("core"),) * n_all,
            out_specs=(PartitionSpec("core"),) * len(out_names),
            check_rep=False,
        ),
        keep_unused=True,
    )
    return sharded, mesh, in_names, out_names, zero_outs


def bench(weight_blend, x, W0, B0, W1, B1, W2, B2, iters=20):
    """Time the kernel two ways: per-dispatch (reps=1) and in-program repeat
    slope ((T_R - T_1)/(R-1)) which cancels dispatch overhead.
    Returns (output, slope_seconds)."""
    import time as _time

    import jax
    from jax.sharding import NamedSharding, PartitionSpec

    in_maps = _prep_in_maps(weight_blend, x, W0, B0, W1, B1, W2, B2)
    mode = _mm_mode()

    N = int(os.environ.get("MOE_HWLOOP", "201"))
    R = int(os.environ.get("MOE_REPS", "4"))
    nc1 = _build_program(mm_mode=mode, reps=R, hw_loop=1)
    sharded1, mesh, in_names, out_names, zero_outs = _make_sharded_fn(nc1)
    ncR = _build_program(mm_mode=mode, reps=R, hw_loop=N)
    shardedR, _, _, _, _ = _make_sharded_fn(ncR)

    spec = NamedSharding(mesh, PartitionSpec("core"))
    args = []
    for name in in_names:
        concat = np.concatenate([in_maps[c][name] for c in range(NCORES)], axis=0)
        args.append(jax.device_put(concat, spec))
    for z in zero_outs:
        concat = np.concatenate([z] * NCORES, axis=0)
        args.append(jax.device_put(concat, spec))

    def timeit(fn):
        outs = fn(*args)
        jax.block_until_ready(outs)
        for _ in range(3):
            outs = fn(*args)
        jax.block_until_ready(outs)
        times = []
        for _ in range(iters):
            t0 = _time.perf_counter()
            outs = fn(*args)
            jax.block_until_ready(outs)
            times.append(_time.perf_counter() - t0)
        times = np.asarray(times)
        return float(np.median(times)), float(times.min()), outs

    t1_med, t1_min, outs = timeit(sharded1)
    tR_med, tR_min, _ = timeit(shardedR)
    slope = (tR_med - t1_med) / ((N - 1) * R)
    slope_min = (tR_min - t1_min) / ((N - 1) * R)
    print(f"sync per-call hwloop=1: med {t1_med * 1e6:.1f} min {t1_min * 1e6:.1f} us")
    print(f"sync per-call hwloop={N}: med {tR_med * 1e6:.1f} min {tR_min * 1e6:.1f} us")
    print(f"kernel slope: med {slope * 1e6:.1f} min {slope_min * 1e6:.1f} us")

    yt = np.asarray(outs[out_names.index("yT")]).reshape(NCORES, DIMS[3], B)
    out = np.concatenate(list(yt), axis=1)
    return np.ascontiguousarray(out.T, dtype=np.float32), slope


# revision 11
# speedup vs baseline: 1.4961x; 1.4961x over previous
"""Blended-expert MoE MLP (moe_routing) Trainium2 Bass kernel.

Math per layer l:  t[b,o] = sum_e wb[b,e] * (W_l[e] @ x[b] + B_l[e])
                   x_next = elu(t)   (layers 0,1; layer 2 linear)

Reformulated as one GEMM per layer with contraction k = (i_tile, e, p):
    t[o, b] = sum_k Wp[k, o] * xp[k, b]
where xp[(i_tile,e,p), b] = xT[i_tile*128+p, b] * wbT[e, b]  (built on-chip
by DVE) and the bias enters as an extra K=8 matmul with rhs = wbT directly.

v2: skewed per-output-pair contraction. Weights are laid out host-side as
[o_pair j][k-chunk c][g][p][256 o] so each 512KB chunk is contiguous in DRAM
and streams in exactly the order the PE consumes it. Each o-pair's PSUM
accumulation closes after its KT k-tiles, so evict+ELU+next-layer xp build
overlap the next pair's matmuls, and layer l+1's matmuls overlap layer l's
tail pairs (cross-layer software pipelining via Tile dependency scheduling).
PSUM tiles are padded to a full 2KB bank each to avoid bank-sharing hazards.

Everything on-device is feature-major ([feature, batch]) so each layer's
PSUM output [o, b] is directly the next layer's input layout.

Sharding: data-parallel over batch: 2048 -> 8 cores x 256. Weights are
replicated (streamed from HBM each layer, ~33MB/core in f16).
"""

import os
import sys

import numpy as np

if not any("trn_rl_repo" in p for p in sys.path):
    sys.path.append("/opt/trn_rl_repo")

from concourse import bacc, mybir  # noqa: E402
import concourse.bass as bass  # noqa: E402
import concourse.tile as tile  # noqa: E402

F32 = mybir.dt.float32
F16 = mybir.dt.float16


def _mm_mode():
    return os.environ.get("MOE_MM_DTYPE", "f16")


E = 8
DIMS = [512, 1024, 1024, 512]
BATCH = 2048
NCORES = 8
B = BATCH // NCORES  # 256 per-core batch
P = 128
OW = 512  # o-columns per group (4 PSUM tiles; 1KB DMA runs)
TPG = OW // P  # tiles per group

NI = [DIMS[0] // P, DIMS[1] // P, DIMS[2] // P]  # [4, 8, 8] input tiles / layer
NO = [DIMS[1] // P, DIMS[2] // P, DIMS[3] // P]  # [8, 8, 4] output tiles / layer
NJ = [DIMS[1] // OW, DIMS[2] // OW, DIMS[3] // OW]  # [2, 2, 1] o-groups / layer
KT = [NI[l] * E for l in range(3)]  # [32, 64, 64] contraction tiles / layer
G = 4  # k-tiles per streamed weight chunk

_CACHE = {}


def _build_program(mm_mode: str = "f16", reps: int = 1, hw_loop: int = 0):
    """Build (and cache) the Bass program. Same program runs SPMD on all cores.
    reps>1 unrolls the whole computation in-program; hw_loop>0 wraps it in a
    hardware For_i loop (for timing-slope measurements that cancel out
    per-dispatch overhead)."""
    key = ("prog", mm_mode, reps, hw_loop)
    if key in _CACHE:
        return _CACHE[key]
    assert mm_mode == "f16", "v2 kernel supports f16 only"

    nc = bacc.Bacc("TRN2", target_bir_lowering=False, debug=False, num_devices=NCORES)

    xT_d = nc.dram_tensor("xT", [DIMS[0], B], F32, kind="ExternalInput")
    wbT16_d = nc.dram_tensor("wbT16", [E, B], F16, kind="ExternalInput")
    wp_d = [
        nc.dram_tensor(f"Wp{l}", [NJ[l] * KT[l] * P, OW], F16, kind="ExternalInput")
        for l in range(3)
    ]
    wb_d = [
        nc.dram_tensor(f"Wb{l}", [E, DIMS[l + 1]], F16, kind="ExternalInput")
        for l in range(3)
    ]
    yT_d = nc.dram_tensor("yT", [DIMS[3], B], F32, kind="ExternalOutput")

    stag = os.environ.get("MOE_STAGGER", "0") == "1"

    with tile.TileContext(nc) as tc:
        with (
            tc.tile_pool(name="const", bufs=1) as const_pool,
            tc.tile_pool(name="xpool", bufs=2) as x_pool,
            tc.tile_pool(name="xppool", bufs=2) as xp_pool,
            tc.tile_pool(name="wstream", bufs=6) as w_pool,
            tc.tile_pool(name="tmp", bufs=8) as tmp_pool,
            tc.tile_pool(name="psum", bufs=8, space="PSUM") as psum_pool,
        ):
            # ---- loop-invariant constants: blend rows, blend broadcast, biases ----
            wb_sb = const_pool.tile([E, B], F16)
            nc.sync.dma_start(wb_sb[:], wbT16_d[:])

            # wb broadcast to all 128 partitions: [128, E, B] (f16, 512KB)
            wb_bc = const_pool.tile([P, E, B], F16)
            nc.sync.dma_start(
                wb_bc[:],
                wbT16_d.rearrange("e b -> (e b)")
                .unsqueeze(0)
                .partition_broadcast(P)
                .squeeze(1)
                .rearrange("p (e b) -> p e b", e=E),
            )

            wbias = []
            for l in range(3):
                wbl = const_pool.tile([E, DIMS[l + 1]], F16, name=f"wbias_{l}")
                nc.sync.dma_start(wbl[:], wb_d[l][:])
                wbias.append(wbl)

            # x0 / xp0 are software-pipelined one iteration ahead: loaded and
            # built here for the first iteration, then re-loaded/rebuilt at
            # each iteration's tail (when DMA + DVE are otherwise idle) so the
            # PE can start layer 0 immediately after the loop barrier.
            x0_sb = const_pool.tile([P, NI[0], B], F32, name="x0")
            xp0_sb = const_pool.tile([P, KT[0], B], F16, name="xp0")

            def load_x0_build_xp0():
                h = NI[0] // 2
                nc.sync.dma_start(
                    x0_sb[:, :h, :],
                    xT_d[: h * P, :].rearrange("(t p) b -> p t b", p=P),
                )
                nc.sync.dma_start(
                    x0_sb[:, h:, :],
                    xT_d[h * P :, :].rearrange("(t p) b -> p t b", p=P),
                )
                for it in range(NI[0]):
                    nc.vector.tensor_tensor(
                        out=xp0_sb[:, it * E : (it + 1) * E, :],
                        in0=x0_sb[:, it : it + 1, :].broadcast_to([P, E, B]),
                        in1=wb_bc[:],
                        op=mybir.AluOpType.mult,
                    )

            load_x0_build_xp0()

            import contextlib

            loop_cm = (
                tc.For_i(0, hw_loop, 1, staggered_reset=stag)
                if hw_loop > 0
                else contextlib.nullcontext()
            )
            with loop_cm:
              for rep in range(reps):
                xp = xp0_sb
                wdma_n = 0
                for l in range(3):
                    nI, nJ, O = NI[l], NJ[l], DIMS[l + 1]
                    ktl = KT[l]
                    nchunk = ktl // G

                    if l < 2:
                        x_next = x_pool.tile(
                            [P, NO[l], B], F32, tag="x", name=f"x{l + 1}_{rep}"
                        )
                        xp_next = xp_pool.tile(
                            [P, KT[2], B], F16, tag="xp", name=f"xp{l + 1}_{rep}"
                        )
                    else:
                        x_next = x_pool.tile(
                            [P, NO[l], B], F32, tag="x", name=f"y_{rep}"
                        )
                        xp_next = None

                    for j in range(nJ):
                        # PSUM accumulators, one full bank per o-tile
                        po = []
                        for t in range(TPG):
                            po_t = psum_pool.tile(
                                [P, B], F32, tag="po",
                                name=f"po_{l}_{j}_{t}_{rep}",
                                padded_shape=[P, 512],
                            )
                            po.append(po_t)
                            # bias matmul opens the accumulation group
                            nc.tensor.matmul(
                                po_t[:],
                                wbias[l][:, j * OW + t * P : j * OW + (t + 1) * P],
                                wb_sb[:],
                                start=True,
                                stop=False,
                            )

                        # stream this group's weights in G-k-tile chunks,
                        # alternating the issuing HWDGE engine (SP / Activation)
                        for c in range(nchunk):
                            w_sb = w_pool.tile(
                                [P, G, OW], F16, tag="w", name=f"w_{l}_{j}_{c}_{rep}"
                            )
                            base = (j * ktl + c * G) * P
                            eng = nc.sync if (wdma_n % 2 == 0) else nc.scalar
                            wdma_n += 1
                            eng.dma_start(
                                w_sb[:],
                                wp_d[l][base : base + G * P, :].rearrange(
                                    "(g p) o -> p g o", p=P
                                ),
                            )
                            for g in range(G):
                                kt = c * G + g
                                last = kt == ktl - 1
                                for t in range(TPG):
                                    nc.tensor.matmul(
                                        po[t][:],
                                        w_sb[:, g, t * P : (t + 1) * P],
                                        xp[:, kt, :],
                                        start=False,
                                        stop=last,
                                    )

                        # ---- evict + ELU, then build next layer's xp slices ----
                        for t in range(TPG):
                            ot = TPG * j + t
                            if l < 2:
                                # elu(t) = (min(exp(t),1) - 1) + max(t, 0)
                                ex = tmp_pool.tile(
                                    [P, B], F32, tag="ex", name=f"ex_{l}_{ot}_{rep}"
                                )
                                nc.scalar.activation(
                                    ex[:], po[t][:], mybir.ActivationFunctionType.Exp
                                )
                                em1 = tmp_pool.tile(
                                    [P, B], F32, tag="em1", name=f"em1_{l}_{ot}_{rep}"
                                )
                                nc.vector.tensor_scalar(
                                    em1[:],
                                    ex[:],
                                    1.0,
                                    -1.0,
                                    op0=mybir.AluOpType.min,
                                    op1=mybir.AluOpType.add,
                                )
                                nc.vector.scalar_tensor_tensor(
                                    x_next[:, ot, :],
                                    po[t][:],
                                    0.0,
                                    em1[:],
                                    op0=mybir.AluOpType.max,
                                    op1=mybir.AluOpType.add,
                                )
                                # next layer xp slice (needs only this x tile)
                                nc.vector.tensor_tensor(
                                    out=xp_next[:, ot * E : (ot + 1) * E, :],
                                    in0=x_next[:, ot : ot + 1, :].broadcast_to(
                                        [P, E, B]
                                    ),
                                    in1=wb_bc[:],
                                    op=mybir.AluOpType.mult,
                                )
                            else:
                                # final layer: copy (no ELU), alternate engines,
                                # store each tile as soon as it's evicted
                                if t % 2 == 0:
                                    nc.vector.tensor_copy(x_next[:, ot, :], po[t][:])
                                else:
                                    nc.scalar.activation(
                                        x_next[:, ot, :],
                                        po[t][:],
                                        mybir.ActivationFunctionType.Copy,
                                    )
                                nc.sync.dma_start(
                                    yT_d[ot * P : (ot + 1) * P, :],
                                    x_next[:, ot, :],
                                )

                    if l < 2:
                        x_sb = x_next
                        xp = xp_next

                # prefetch next iteration's x0 / xp0 while DMA+DVE are idle
                load_x0_build_xp0()

    nc.compile()
    _CACHE[key] = nc
    return nc


def _prep_weights(W, l):
    """Rearrange (E, O, I) weights into the streamed layout:
    [j, kt, p, o256] with kt = i_tile*E + e, flattened to
    [nJ*KT*128, 256]; element (j, it*E+e, p, o) = W[e, j*256+o, it*128+p]."""
    O, I = DIMS[l + 1], DIMS[l]
    nI, nJ = I // P, O // OW
    out = (
        W.reshape(E, nJ, OW, nI, P)
        .transpose(1, 3, 0, 4, 2)
        .reshape(nJ * nI * E * P, OW)
    )
    return np.ascontiguousarray(out, dtype=np.float16)


def _prep_in_maps(weight_blend, x, W0, B0, W1, B1, W2, B2):
    weight_blend = np.asarray(weight_blend, dtype=np.float32)
    x = np.asarray(x, dtype=np.float32)
    Ws = [np.asarray(w, dtype=np.float32) for w in (W0, W1, W2)]
    Bs = [np.asarray(b, dtype=np.float32) for b in (B0, B1, B2)]
    wp = [_prep_weights(Ws[l], l) for l in range(3)]
    wbias = [np.ascontiguousarray(Bs[l][:, :, 0], dtype=np.float16) for l in range(3)]
    in_maps = []
    for c in range(NCORES):
        sl = slice(c * B, (c + 1) * B)
        in_maps.append(
            {
                "xT": np.ascontiguousarray(x[sl].T),
                "wbT16": np.ascontiguousarray(weight_blend[sl].T, dtype=np.float16),
                "Wp0": wp[0],
                "Wp1": wp[1],
                "Wp2": wp[2],
                "Wb0": wbias[0],
                "Wb1": wbias[1],
                "Wb2": wbias[2],
            }
        )
    return in_maps


def kernel(weight_blend, x, W0, B0, W1, B1, W2, B2):
    from concourse.bass_utils import run_bass_kernel_spmd

    in_maps = _prep_in_maps(weight_blend, x, W0, B0, W1, B1, W2, B2)
    nc = _build_program(mm_mode=_mm_mode())
    res = run_bass_kernel_spmd(nc, in_maps, list(range(NCORES)))
    out = np.concatenate([res.results[c]["yT"] for c in range(NCORES)], axis=1)
    return np.ascontiguousarray(out.T, dtype=np.float32)


def _make_sharded_fn(nc):
    """Build the shard_map'd jitted executable, mirroring
    bass2jax.run_bass_via_pjrt's multi-core path but without output donation
    so it can be re-invoked for timing."""
    import jax
    from jax.experimental.shard_map import shard_map
    from jax.sharding import Mesh, PartitionSpec
    from concourse import bass2jax, mybir as _mybir

    bass2jax.install_neuronx_cc_hook()

    partition_name = nc.partition_id_tensor.name if nc.partition_id_tensor else None
    in_names, out_names, out_avals, zero_outs = [], [], [], []
    for alloc in nc.m.functions[0].allocations:
        if not isinstance(alloc, _mybir.MemoryLocationSet):
            continue
        name = alloc.memorylocations[0].name
        if alloc.kind == "ExternalInput":
            if name != partition_name:
                in_names.append(name)
        elif alloc.kind == "ExternalOutput":
            out_names.append(name)
            shape = tuple(alloc.tensor_shape)
            dtype = _mybir.dt.np(alloc.dtype)
            out_avals.append(jax.core.ShapedArray(shape, dtype))
            zero_outs.append(np.zeros(shape, dtype))
    n_params = len(in_names)
    all_names = in_names + out_names
    if partition_name is not None:
        all_names = all_names + [partition_name]

    def _body(*args):
        operands = list(args)
        if partition_name is not None:
            operands.append(bass2jax.partition_id_tensor())
        outs = bass2jax._bass_exec_p.bind(
            *operands,
            out_avals=tuple(out_avals),
            in_names=tuple(all_names),
            out_names=tuple(out_names),
            lowering_input_output_aliases=(),
            sim_require_finite=True,
            sim_require_nnan=True,
            nc=nc,
        )
        return tuple(outs)

    devices = jax.devices()[:NCORES]
    mesh = Mesh(np.asarray(devices), ("core",))
    n_all = n_params + len(out_names)
    sharded = jax.jit(
        shard_map(
            _body,
            mesh=mesh,
            in_specs=(PartitionSpec("core"),) * n_all,
            out_specs=(PartitionSpec("core"),) * len(out_names),
            check_rep=False,
        ),
        keep_unused=True,
    )
    return sharded, mesh, in_names, out_names, zero_outs


def bench(weight_blend, x, W0, B0, W1, B1, W2, B2, iters=20):
    """Time the kernel two ways: per-dispatch (reps=1) and in-program repeat
    slope ((T_R - T_1)/(R-1)) which cancels dispatch overhead.
    Returns (output, slope_seconds)."""
    import time as _time

    import jax
    from jax.sharding import NamedSharding, PartitionSpec

    in_maps = _prep_in_maps(weight_blend, x, W0, B0, W1, B1, W2, B2)
    mode = _mm_mode()

    N = int(os.environ.get("MOE_HWLOOP", "26"))
    R = int(os.environ.get("MOE_REPS", "4"))
    nc1 = _build_program(mm_mode=mode, reps=R, hw_loop=1)
    sharded1, mesh, in_names, out_names, zero_outs = _make_sharded_fn(nc1)
    ncR = _build_program(mm_mode=mode, reps=R, hw_loop=N)
    shardedR, _, _, _, _ = _make_sharded_fn(ncR)

    spec = NamedSharding(mesh, PartitionSpec("core"))
    args = []
    for name in in_names:
        concat = np.concatenate([in_maps[c][name] for c in range(NCORES)], axis=0)
        args.append(jax.device_put(concat, spec))
    for z in zero_outs:
        concat = np.concatenate([z] * NCORES, axis=0)
        args.append(jax.device_put(concat, spec))

    def timeit(fn):
        outs = fn(*args)
        jax.block_until_ready(outs)
        for _ in range(3):
            outs = fn(*args)
        jax.block_until_ready(outs)
        times = []
        for _ in range(iters):
            t0 = _time.perf_counter()
            outs = fn(*args)
            jax.block_until_ready(outs)
            times.append(_time.perf_counter() - t0)
        times = np.asarray(times)
        return float(np.median(times)), float(times.min()), outs

    t1_med, t1_min, outs = timeit(sharded1)
    tR_med, tR_min, _ = timeit(shardedR)
    slope = (tR_med - t1_med) / ((N - 1) * R)
    slope_min = (tR_min - t1_min) / ((N - 1) * R)
    print(f"sync per-call hwloop=1: med {t1_med * 1e6:.1f} min {t1_min * 1e6:.1f} us")
    print(f"sync per-call hwloop={N}: med {tR_med * 1e6:.1f} min {tR_min * 1e6:.1f} us")
    print(f"kernel slope: med {slope * 1e6:.1f} min {slope_min * 1e6:.1f} us")

    yt = np.asarray(outs[out_names.index("yT")]).reshape(NCORES, DIMS[3], B)
    out = np.concatenate(list(yt), axis=1)
    return np.ascontiguousarray(out.T, dtype=np.float32), slope
